# revision 1
# baseline (speedup 1.0000x reference)
"""BandSplit kernel for Trainium2 (8 NeuronCores, SPMD data-parallel over batch).

Reference computation (per band i, band width b, c=2b):
    xb[b,t,c]   = x[b, f0:f0+b, t, :] transposed/reshaped     (B, T, c)
    GroupNorm(1, c) over (T, c) per sample, affine gn_w/gn_b
    Linear(c -> 128) with fc_w/fc_b
    out stacked over 31 bands -> [B, T, 128, 31]

Key algebra: the whole band op is affine in x per sample:
    z[t,o] = s * sum_c x[t,c] * (gn_w[c]*fc_w[o,c])
             + (beta[o] + (-mu*s) * g[o])
  with s = rsqrt(var+eps), beta = fc_b + fc_w@gn_b, g = fc_w@gn_w.
The bias enters as two extra "ones" channels in the contraction, so one
matmul per band produces the final output block.
"""

import os
import numpy as np

import concourse.bass as bass
import concourse.tile as tile
import concourse.mybir as mybir
from concourse.bass_utils import run_bass_kernel_spmd

# ----------------------------------------------------------------------------
# Problem constants (hardcoded; kernel.py must be self-contained)
# ----------------------------------------------------------------------------
BANDS = [2, 3, 3, 3, 3, 3, 3, 3, 3, 3, 3, 8, 8, 8, 8, 8, 8, 8, 8, 8, 8, 8, 8,
         16, 16, 16, 16, 16, 16, 16, 17]
NB = len(BANDS)           # 31
CH = 128                  # output channels per band
MAX_C = 34
EPS = 1e-5
B_FULL, F, T = 16, 257, 1000
N_CORES = 8
B_LOC = B_FULL // N_CORES  # 2 samples per core

# matmul input dtype: "f32" (safe, 4 cyc/col) or "f16" (fast, 1 cyc/col)
MM_DT = os.environ.get("BS_MM_DT", "f32")

# t-chunks of the main loop
CHUNKS = [(t0, min(128, T - t0)) for t0 in range(0, T, 128)]

# f-tiles for the raw input (natural layout)
FT = [(0, 128), (128, 112), (240, 17)]
# bands covered by each f-tile (band f-ranges align with these splits)
FT_BANDS = [(0, 23), (23, 30), (30, 31)]

# Dense per-tile packing. All matmuls use base partition 0 (PE tile_position
# other than (0,0) crashes the device runtime); a group matmul contracts over
# rows [0, group_end) of its tile with zero weights on rows of other bands.
TILE_BANDS = [(0, 13), (13, 20), (20, 25), (25, 28), (28, 31)]
N_XT = 5

# groups of <=4 bands per matmul (N = 128*nb <= 512); psum col is bank-aligned
GROUP_BANDS = [(0, 4), (4, 8), (8, 11), (11, 13),
               (13, 17), (17, 20),
               (20, 23), (23, 25),
               (25, 28),
               (28, 31)]
# (psum_idx, col): psum tensor A/B/C and column offset of each group
GROUP_PSUM = [(0, 0), (0, 512), (0, 1024), (0, 1536),
              (1, 0), (1, 512),
              (2, 0), (2, 512),  # placeholder, fixed below
              (2, 0), (2, 0)]
GROUP_PSUM = [(0, 0), (0, 512), (0, 1024), (0, 1536),
              (1, 0), (1, 512), (1, 1024), (1, 1536),
              (2, 0), (2, 512)]
# drain runs: (psum_idx, band_lo, band_hi, psum_col); bands within a run sit
# at uniform 128-col stride in their psum tensor
DRAIN_RUNS = [(0, 0, 11, 0), (0, 11, 13, 1536),
              (1, 13, 20, 0), (1, 20, 23, 1024), (1, 23, 25, 1536),
              (2, 25, 28, 0), (2, 28, 31, 512)]


def _band_info():
    """band i -> (tile, row_start, b, f0); rows per band = 2 + 2b."""
    info = []
    f0 = 0
    for t, (lo, hi) in enumerate(TILE_BANDS):
        r = 0
        for i in range(lo, hi):
            b = BANDS[i]
            info.append((t, r, b, f0))
            r += 2 + 2 * b
            f0 += b
    info.sort(key=lambda e: 0)  # already in band order by construction
    # reorder to band order
    out = [None] * NB
    f0 = 0
    idx = 0
    for t, (lo, hi) in enumerate(TILE_BANDS):
        r = 0
        for i in range(lo, hi):
            b = BANDS[i]
            out[i] = (t, r, b, sum(BANDS[:i]))
            r += 2 + 2 * b
    return out

INFO = _band_info()
TILE_ROWS = [sum(2 + 2 * BANDS[i] for i in range(lo, hi))
             for (lo, hi) in TILE_BANDS]
# weight-tile column layout: band i occupies cols [loc_i*128, (loc_i+1)*128)
# of its tile's wt tensor; global P1 col offset per tile:
WT_COLS = [128 * (hi - lo) for (lo, hi) in TILE_BANDS]
WT_OFF = [sum(WT_COLS[:t]) for t in range(N_XT)]
TOT_COLS = sum(WT_COLS)  # 3968


def _build_const_tables(gn_w, gn_b, fc_w, fc_b):
    """Host-side packing of the (tiny) parameters into matmul-ready tables."""
    # P1: [128, 3968] block-diagonal weights; band i of tile t sits at
    # rows [r, r+2+2b) and cols [WT_OFF[t] + loc*128, ... + (loc+1)*128)
    p1 = np.zeros((128, TOT_COLS), np.float32)
    for i, (t, r, b, f0) in enumerate(INFO):
        c = 2 * b
        loc = i - TILE_BANDS[t][0]
        col = WT_OFF[t] + loc * CH
        w = fc_w[i, :, :c].astype(np.float64)          # [128, c]
        beta = fc_b[i] + w @ gn_b[i, :c]               # [128]
        g = w @ gn_w[i, :c]                            # [128]
        w2 = (w * gn_w[i, :c][None, :]).T              # [c, 128]
        p1[r + 0, col:col + CH] = beta
        p1[r + 1, col:col + CH] = g
        p1[r + 2: r + 2 + b, col:col + CH] = w2[0::2]        # E rows
        p1[r + 2 + b: r + 2 + 2 * b, col:col + CH] = w2[1::2]  # O rows

    # Msel: [63, N_XT*128]; C_col = Msel^T @ vec63
    # vec63 = [s_0..s_30, (-mu*s)_0..30, 1.0]
    msel = np.zeros((63, N_XT * 128), np.float32)
    for i, (t, r, b, f0) in enumerate(INFO):
        col = t * 128 + r
        msel[62, col + 0] = 1.0            # beta row: C=1
        msel[31 + i, col + 1] = 1.0        # g row: C=-mu*s
        msel[i, col + 2: col + 2 + 2 * b] = 1.0  # channel rows: C=s

    # Ind: [257, 31] band indicator over f rows
    ind = np.zeros((F, NB), np.float32)
    for i, (t, st, b, f0) in enumerate(INFO):
        ind[f0:f0 + b, i] = 1.0

    # invCT2: [1, 62] = 1 / (c_i * T), duplicated for the Sx and Sxx halves
    invct = np.array([1.0 / (2 * b * T) for b in BANDS], np.float32)
    invct2 = np.concatenate([invct, invct])[None, :]

    ones8 = np.ones((8, 1000), np.float32)
    return p1, msel, ind, invct2, ones8


# ----------------------------------------------------------------------------
# Bass kernel
# ----------------------------------------------------------------------------
_NC_CACHE = {}


def _spill_waits(nc):
    """Split multi-wait instructions into NoOp(wait) + instruction.

    The walrus build in this container enforces the HW wait capacity
    (1 sync wait per instruction, 2 for EventSemaphore); Tile emits more.
    Engine queues are in-order, so hoisting extra waits into preceding
    NoOps on the same queue preserves semantics.
    """
    n = 0
    for fn in nc.m.functions:
        for bb in fn.blocks:
            out = []
            changed = False
            for inst in bb.instructions:
                si = getattr(inst, "sync_info", None)
                cap = 2 if isinstance(inst, mybir.InstEventSemaphore) else 1
                if si is not None and si.on_wait and len(si.on_wait) > cap:
                    waits = list(si.on_wait)
                    extra, keep = waits[:-cap], waits[-cap:]
                    for w in extra:
                        nop = mybir.InstNoOp(name=f"{inst.name}_w{n}",
                                             ins=[], outs=[])
                        nop.engine = inst.engine
                        nop.sync_info = mybir.SyncInfo(on_wait=[w],
                                                       on_update=[])
                        out.append(nop)
                        n += 1
                    si.on_wait = keep
                    changed = True
                out.append(inst)
            if changed:
                bb.instructions = out
    return n


def _mm_dt():
    return mybir.dt.float32 if MM_DT == "f32" else mybir.dt.float16


def build_bass():
    repeat = int(os.environ.get("BS_REPEAT", "1"))
    key = (MM_DT, repeat)
    if key in _NC_CACHE:
        return _NC_CACHE[key]
    DT = _mm_dt()
    F32 = mybir.dt.float32

    nc = bass.Bass("TRN2", target_bir_lowering=False, debug=False,
                   num_devices=N_CORES)

    x_d = nc.dram_tensor("x", [B_LOC, F, T, 2], F32, kind="ExternalInput").ap()
    p1_d = nc.dram_tensor("p1", [128, TOT_COLS], F32, kind="ExternalInput").ap()
    msel_d = nc.dram_tensor("msel", [63, N_XT * 128], F32, kind="ExternalInput").ap()
    ind_d = nc.dram_tensor("ind", [F, NB], F32, kind="ExternalInput").ap()
    invct_d = nc.dram_tensor("invct2", [1, 2 * NB], F32, kind="ExternalInput").ap()
    ones_d = nc.dram_tensor("ones8", [8, 1000], F32, kind="ExternalInput").ap()
    z_d = nc.dram_tensor("z", [B_LOC, T, CH, NB], F32, kind="ExternalOutput").ap()

    AluOp = mybir.AluOpType
    ActFn = mybir.ActivationFunctionType

    with tile.TileContext(nc) as tc:
        with (
            tc.tile_pool(name="const", bufs=1) as constp,
            tc.tile_pool(name="a", bufs=2) as ap_,
            tc.tile_pool(name="eo", bufs=4) as eop,
            tc.tile_pool(name="xg", bufs=1) as xgp,
            tc.tile_pool(name="wt", bufs=1) as wtp,
            tc.tile_pool(name="small", bufs=8) as smp,
            tc.tile_pool(name="out", bufs=(2 if MM_DT == "f32" else 3)) as outp,
            tc.tile_pool(name="sq", bufs=1) as sqp,
            tc.tile_pool(name="psum", bufs=2, space="PSUM") as psp,
        ):
            # ---------------- constants to SBUF ----------------
            p1_sb = []
            for t in range(N_XT):
                pt = constp.tile([128, WT_COLS[t]], F32, tag=f"p1_{t}",
                                 name=f"p1c_{t}")
                nc.sync.dma_start(
                    pt[:], p1_d[:, WT_OFF[t]:WT_OFF[t] + WT_COLS[t]])
                p1_sb.append(pt)
            msel_sb = constp.tile([63, N_XT * 128], F32, tag="msel")
            nc.sync.dma_start(msel_sb[:], msel_d[:])
            ind_sb = []
            for g, (f0, P) in enumerate(FT):
                it = constp.tile([P, NB], F32, tag=f"ind_{g}", name=f"indc_{g}")
                nc.sync.dma_start(it[:], ind_d[f0:f0 + P, :])
                ind_sb.append(it)
            invct_sb = constp.tile([1, 2 * NB], F32, tag="invct")
            nc.sync.dma_start(invct_sb[:], invct_d[:])
            ident = constp.tile([1, 1], F32, tag="ident")
            nc.vector.memset(ident[:], 1.0)
            zcol = constp.tile([128, 1], F32, tag="zcol")
            nc.vector.memset(zcol[:], 0.0)
            epsc = constp.tile([1, 1], F32, tag="epsc")
            nc.vector.memset(epsc[:], EPS)

            # persistent per-sample tiles
            xg = [[xgp.tile([128, T], DT, tag=f"xg_{s}_{t}", name=f"xg_{s}_{t}")
                   for t in range(N_XT)] for s in range(B_LOC)]
            wt = [[wtp.tile([128, WT_COLS[t]], DT, tag=f"wt_{s}_{t}",
                            name=f"wt_{s}_{t}")
                   for t in range(N_XT)] for s in range(B_LOC)]

            # ---------------- body (repeatable for benchmarking) ------
            for _rep in range(repeat):
              # ---------------- prologue per sample ----------------
              for s in range(B_LOC):
                  # ones rows (beta-row + g-row per band)
                  for i, (t, r, b, f0) in enumerate(INFO):
                      nc.scalar.dma_start(xg[s][t][r:r + 2, :], ones_d[0:2, :])

                  mom = psp.tile([1, 2 * NB], F32, tag="main")
                  for g, (f0, P) in enumerate(FT):
                      A = ap_.tile([P, 2000], F32, tag="a")
                      nc.sync.dma_start(
                          A[:], x_d[s, f0:f0 + P].rearrange("p a b -> p (a b)"))
                      stat = smp.tile([P, 2], F32, tag="stat")
                      s1t = smp.tile([P, 2], F32, tag="s1t")
                      # sum of squares via ACT square + accumulate
                      # (scratch out is write-only)
                      Asq = sqp.tile([P, 2000], F32, tag="sq", name="Asq")
                      nc.scalar.activation(Asq[:], A[:], ActFn.Square,
                                           bias=zcol[0:P, :])
                      nc.vector.tensor_reduce(stat[:, 1:2], Asq[:],
                                              mybir.AxisListType.X, AluOp.add)
                      # de-interleave (and cast); accumulate sums per f-row
                      Av = A[:].rearrange("p (t r) -> p r t", r=2)
                      E = eop.tile([P, T], DT, tag="eo")
                      O = eop.tile([P, T], DT, tag="eo")
                      nc.vector.tensor_scalar(E[:], Av[:, 0, :], 1.0, None,
                                              AluOp.mult, AluOp.add,
                                              accum_out=s1t[:, 0:1])
                      nc.vector.tensor_scalar(O[:], Av[:, 1, :], 1.0, None,
                                              AluOp.mult, AluOp.add,
                                              accum_out=s1t[:, 1:2])
                      nc.vector.tensor_tensor(stat[:, 0:1], s1t[:, 0:1],
                                              s1t[:, 1:2], AluOp.add)
                      # per-band moment partial sums -> mom[0, b0:b1] (Sx)
                      # and mom[0, 31+b0:31+b1] (Sxx)
                      b0, b1 = FT_BANDS[g]
                      nc.tensor.matmul(mom[0:1, b0:b1], lhsT=stat[:, 0:1],
                                       rhs=ind_sb[g][:, b0:b1],
                                       start=True, stop=True)
                      nc.tensor.matmul(mom[0:1, NB + b0:NB + b1],
                                       lhsT=stat[:, 1:2],
                                       rhs=ind_sb[g][:, b0:b1],
                                       start=True, stop=True)
                      # remap E/O rows into strip tiles
                      for i in range(b0, b1):
                          t, r, b, f0b = INFO[i]
                          fl = f0b - f0
                          nc.scalar.dma_start(
                              xg[s][t][r + 2: r + 2 + b, :], E[fl:fl + b, :])
                          nc.scalar.dma_start(
                              xg[s][t][r + 2 + b: r + 2 + 2 * b, :],
                              O[fl:fl + b, :])

                  # moments -> s, -mu*s (everything on partition 0, free axis)
                  m2 = smp.tile([1, 2 * NB], F32, tag="m2")
                  nc.vector.tensor_tensor(m2[:], mom[:], invct_sb[:],
                                          AluOp.mult)   # [mu | ex2]
                  mu = m2[:, 0:NB]
                  ex2 = m2[:, NB:2 * NB]
                  var = smp.tile([1, NB], F32, tag="var")
                  nc.vector.tensor_tensor(var[:], mu, mu, AluOp.mult)  # mu^2
                  nc.vector.tensor_tensor(var[:], ex2, var[:],
                                          AluOp.subtract)   # ex2 - mu^2
                  sd = smp.tile([1, NB], F32, tag="sd")
                  nc.scalar.activation(sd[:], var[:], ActFn.Sqrt,
                                       bias=epsc[:])
                  vrow = smp.tile([1, 64], F32, tag="vrow")
                  nc.vector.reciprocal(vrow[:, 0:NB], sd[:])         # s
                  tmp = smp.tile([1, NB], F32, tag="tmp")
                  nc.vector.tensor_tensor(tmp[:], mu, vrow[:, 0:NB],
                                          AluOp.mult)       # mu*s
                  nc.vector.tensor_scalar(vrow[:, NB:2 * NB], tmp[:], -1.0, None,
                                          AluOp.mult)       # -mu*s
                  nc.vector.memset(vrow[:, 62:63], 1.0)

                  v63p = psp.tile([63, 1], F32, tag="main")
                  nc.tensor.transpose(v63p[:], vrow[:, 0:63], ident[:])
                  v63 = smp.tile([63, 1], F32, tag="v63")
                  nc.vector.tensor_copy(v63[:], v63p[:])

                  cvp = psp.tile([128, N_XT], F32, tag="main")
                  for t in range(N_XT):
                      nc.tensor.matmul(cvp[:, t:t + 1],
                                       lhsT=msel_sb[:, t * 128:(t + 1) * 128],
                                       rhs=v63[:], start=True, stop=True)
                  csb = smp.tile([128, N_XT], F32, tag="csb")
                  nc.vector.tensor_copy(csb[:], cvp[:])
                  for t in range(N_XT):
                      nc.vector.tensor_scalar(wt[s][t][:], p1_sb[t][:],
                                              csb[:, t:t + 1], None, AluOp.mult)

              # ---------------- main loop ----------------
              n_chunks = int(os.environ.get("BS_NCHUNKS", "8"))
              skip_mm = os.environ.get("BS_SKIP_MM") == "1"
              skip_drain = os.environ.get("BS_SKIP_DRAIN") == "1"
              skip_out = os.environ.get("BS_SKIP_OUT") == "1"
              for s in range(B_LOC):
                  for (t0, M) in CHUNKS[:n_chunks]:
                      ob = outp.tile([128, CH * NB], F32, tag="ob")
                      ob_v = ob[0:M].rearrange("p (o i) -> p i o", o=CH, i=NB)
                      ps = []
                      for pi in range(3):
                          pt = psp.tile([128, 2048], F32, tag="main",
                                        name=f"ps{pi}")
                          ps.append(pt)
                          for gi, (blo, bhi) in enumerate(GROUP_BANDS):
                              if GROUP_PSUM[gi][0] != pi:
                                  continue
                              col = GROUP_PSUM[gi][1]
                              t = INFO[blo][0]
                              rend = (INFO[bhi - 1][1] + 2
                                      + 2 * INFO[bhi - 1][2])
                              lloc = blo - TILE_BANDS[t][0]
                              wcol = lloc * CH
                              n = (bhi - blo) * CH
                              if not skip_mm:
                                  nc.tensor.matmul(
                                      pt[0:M, col:col + n],
                                      lhsT=xg[s][t][0:rend, t0:t0 + M],
                                      rhs=wt[s][t][0:rend, wcol:wcol + n],
                                      start=True, stop=True)
                          # drains for this psum tensor
                          for (dpi, blo, bhi, col) in DRAIN_RUNS:
                              if dpi != pi or skip_drain:
                                  continue
                              n_b = bhi - blo
                              dst = ob_v[:, blo:bhi, :]
                              srcv = pt[0:M, col:col + n_b * CH].rearrange(
                                  "p (i o) -> p i o", i=n_b, o=CH)
                              if pi == 0:
                                  nc.vector.tensor_copy(dst, srcv)
                              else:
                                  nc.scalar.copy(dst, srcv)
                      if not skip_out:
                          nc.sync.dma_start(
                              z_d[s, t0:t0 + M].rearrange("p a b -> p (a b)"),
                              ob[0:M, :])

    _NC_CACHE[key] = nc
    return nc


# ----------------------------------------------------------------------------
# Public entry point
# ----------------------------------------------------------------------------
def kernel(x, gn_w, gn_b, fc_w, fc_b):
    x = np.asarray(x, np.float32)
    gn_w = np.asarray(gn_w, np.float32)
    gn_b = np.asarray(gn_b, np.float32)
    fc_w = np.asarray(fc_w, np.float32)
    fc_b = np.asarray(fc_b, np.float32)

    p1, msel, ind, invct2, ones8 = _build_const_tables(gn_w, gn_b, fc_w, fc_b)
    nc = build_bass()
    if not getattr(nc, "_waits_spilled", False):
        _spill_waits(nc)
        nc._waits_spilled = True

    in_maps = []
    for k in range(N_CORES):
        in_maps.append({
            "x": np.ascontiguousarray(x[k * B_LOC:(k + 1) * B_LOC]),
            "p1": p1, "msel": msel, "ind": ind,
            "invct2": invct2, "ones8": ones8,
        })
    res = run_bass_kernel_spmd(nc, in_maps, core_ids=list(range(N_CORES)))
    z = np.concatenate([r["z"] for r in res.results], axis=0)
    return z



# revision 14
# speedup vs baseline: 2.6819x; 2.6819x over previous
"""BandSplit kernel for Trainium2 (8 NeuronCores, SPMD data-parallel over batch).

Reference computation (per band i, band width b, c=2b):
    xb[b,t,c]   = x[b, f0:f0+b, t, :] transposed/reshaped     (B, T, c)
    GroupNorm(1, c) over (T, c) per sample, affine gn_w/gn_b
    Linear(c -> 128) with fc_w/fc_b
    out stacked over 31 bands -> [B, T, 128, 31]

Algebra: per sample the op is affine in x:
    z[t,o,i] = sum_c (x[t,c]*s_i) * (gn_w[i,c]*fc_w[i,o,c])
               + beta[i,o] + (-mu_i*s_i) * g[i,o]
with s=rsqrt(var+eps), beta=fc_b+fc_w@gn_b, g=fc_w@gn_w.  The s_i scale is
applied to the data rows during the even/odd de-interleave; the weights are
static (packed on host); the bias enters as a per-sample [1, 31*128] row
added during the PSUM->SBUF drains (or pre-loaded into PSUM via a K=1
matmul for banks drained by the scalar engine, which cannot add).

Layout: contraction rows are packed per-tile as [E rows | O rows] so the
de-interleaved data moves into the matmul tiles with 2 slab DMAs per tile.
Weight columns are (o-major, band-minor) interleaved per matmul group so
the drain writes land in runs of nb contiguous output elements.
"""

import os
import numpy as np

import concourse.bass as bass
import concourse.tile as tile
import concourse.mybir as mybir
from concourse.bass_utils import run_bass_kernel_spmd

# ----------------------------------------------------------------------------
# Problem constants (hardcoded; kernel.py must be self-contained)
# ----------------------------------------------------------------------------
BANDS = [2, 3, 3, 3, 3, 3, 3, 3, 3, 3, 3, 8, 8, 8, 8, 8, 8, 8, 8, 8, 8, 8, 8,
         16, 16, 16, 16, 16, 16, 16, 17]
NB = len(BANDS)           # 31
CH = 128                  # output channels per band
EPS = 1e-5
B_FULL, F, T = 16, 257, 1000
N_CORES = 8
B_LOC = B_FULL // N_CORES  # 2 samples per core

F0 = [sum(BANDS[:i]) for i in range(NB)]  # band start freq

# t-chunks of the main loop
CHUNKS = [(t0, min(128, T - t0)) for t0 in range(0, T, 128)]

# f-tiles of the raw input (band ranges align with these splits)
FT = [(0, 128), (128, 112), (240, 17)]
FT_BANDS = [(0, 23), (23, 30), (30, 31)]

# contraction tiles: (band_lo, band_hi, f_start); rows = [E slab | O slab]
TILES = [(0, 11, 0), (11, 19, 32), (19, 23, 96),
         (23, 27, 128), (27, 30, 192), (30, 31, 240)]
N_XT = len(TILES)
ESUM = [sum(BANDS[lo:hi]) for (lo, hi, _) in TILES]   # E-slab rows per tile
TILE_ROWS = [2 * e for e in ESUM]
# which f-tile feeds each contraction tile
TILE_FT = [0, 0, 0, 1, 1, 2]

# matmul groups: (tile, blo, bhi, bank, bankcol, wtcol)
GROUPS = [
    (0, 0, 4, 0, 0, 0),
    (0, 4, 8, 1, 0, 512),
    (0, 8, 11, 2, 0, 1024),
    (1, 11, 15, 3, 0, 0),
    (1, 15, 19, 4, 0, 512),
    (2, 19, 23, 5, 0, 0),
    (3, 23, 27, 6, 0, 0),
    (4, 27, 30, 7, 0, 0),
    (5, 30, 31, 2, 384, 0),
]


def _grp_rend(g):
    t, blo, bhi, _, _, _ = GROUPS[g]
    _, _, fst = TILES[t]
    fend = F0[bhi - 1] + BANDS[bhi - 1]
    return ESUM[t] + (fend - fst)

GREND = [_grp_rend(g) for g in range(len(GROUPS))]

# drain engine per group: "v" vector, "s" scalar (scalar banks get their
# bias pre-loaded into PSUM via a K=1 matmul; gpsimd cannot access PSUM)
DRAIN_ENG = ["v", "v", "v", "v", "v", "s", "s", "s", "v"]
PREFILL_BANKS = sorted({GROUPS[g][3] for g in range(len(GROUPS))
                        if DRAIN_ENG[g] == "s"})


def _build_const_tables(gn_w, gn_b, fc_w, fc_b):
    """Host-side packing of the (tiny) parameters into matmul-ready tables."""
    f16 = np.float16
    # per-tile weight tables [rows_t, 128*nb_t] with (o, band)-interleaved
    # columns per group
    wts = [np.zeros((TILE_ROWS[t], CH * (hi - lo)), np.float32)
           for t, (lo, hi, _) in enumerate(TILES)]
    for gi, (t, blo, bhi, bank, bcol, wtcol) in enumerate(GROUPS):
        _, _, fst = TILES[t]
        nb = bhi - blo
        for il, i in enumerate(range(blo, bhi)):
            b = BANDS[i]
            w = fc_w[i].astype(np.float64)                 # [128, 34]
            for k in range(b):
                floc = F0[i] + k - fst
                cE, cO = 2 * k, 2 * k + 1
                colE = wtcol + np.arange(CH) * nb + il
                wts[t][floc, colE] = gn_w[i, cE] * w[:, cE]
                wts[t][ESUM[t] + floc, colE] = gn_w[i, cO] * w[:, cO]

    # mbias [63, 4096] (bank-order cols): row 62 -> beta, row 31+i -> g
    mbias = np.zeros((63, 4096), np.float32)
    for gi, (t, blo, bhi, bank, bcol, wtcol) in enumerate(GROUPS):
        nb = bhi - blo
        for il, i in enumerate(range(blo, bhi)):
            c = 2 * BANDS[i]
            w = fc_w[i, :, :c].astype(np.float64)          # [128, c]
            beta = fc_b[i] + w @ gn_b[i, :c]               # [128]
            gvec = w @ gn_w[i, :c]                         # [128]
            cols = bank * 512 + bcol + np.arange(CH) * nb + il
            mbias[62, cols] = beta
            mbias[31 + i, cols] = gvec

    # indicator tables over f rows
    ind = np.zeros((F, NB), np.float32)    # for moment matmuls (f32)
    for i in range(NB):
        ind[F0[i]:F0[i] + BANDS[i], i] = 1.0
    indT = np.ascontiguousarray(ind.T).astype(f16)  # [31, 257] for s_frow

    invct = np.array([1.0 / (2 * b * T) for b in BANDS], np.float32)
    invct2 = np.concatenate([invct, invct])[None, :]

    return ([w.astype(f16) for w in wts], mbias.astype(f16), ind,
            indT, invct2)


# ----------------------------------------------------------------------------
# Bass kernel
# ----------------------------------------------------------------------------
_NC_CACHE = {}


def _spill_waits(nc):
    """Split multi-wait instructions into NoOp(wait) + instruction.

    The walrus build in this container enforces the HW wait capacity
    (1 sync wait per instruction, 2 for EventSemaphore); Tile emits more.
    Engine queues are in-order, so hoisting extra waits into preceding
    NoOps on the same queue preserves semantics.
    """
    n = 0
    for fn in nc.m.functions:
        for bb in fn.blocks:
            out = []
            changed = False
            for inst in bb.instructions:
                si = getattr(inst, "sync_info", None)
                cap = 2 if isinstance(inst, mybir.InstEventSemaphore) else 1
                if si is not None and si.on_wait and len(si.on_wait) > cap:
                    waits = list(si.on_wait)
                    extra, keep = waits[:-cap], waits[-cap:]
                    for w in extra:
                        nop = mybir.InstNoOp(name=f"{inst.name}_w{n}",
                                             ins=[], outs=[])
                        nop.engine = inst.engine
                        nop.sync_info = mybir.SyncInfo(on_wait=[w],
                                                       on_update=[])
                        out.append(nop)
                        n += 1
                    si.on_wait = keep
                    changed = True
                out.append(inst)
            if changed:
                bb.instructions = out
    return n


def build_bass():
    repeat = int(os.environ.get("BS_REPEAT", "1"))
    key = (repeat,)
    if key in _NC_CACHE:
        return _NC_CACHE[key]
    F32 = mybir.dt.float32
    F16 = mybir.dt.float16

    nc = bass.Bass("TRN2", target_bir_lowering=False, debug=False,
                   num_devices=N_CORES)

    x_d = nc.dram_tensor("x", [B_LOC, F, T, 2], F32, kind="ExternalInput").ap()
    wt_d = [nc.dram_tensor(f"wt{t}", [TILE_ROWS[t], CH * (hi - lo)], F16,
                           kind="ExternalInput").ap()
            for t, (lo, hi, _) in enumerate(TILES)]
    mbias_d = nc.dram_tensor("mbias", [63, 4096], F16, kind="ExternalInput").ap()
    ind_d = nc.dram_tensor("ind", [F, NB], F32, kind="ExternalInput").ap()
    indT_d = nc.dram_tensor("indT", [NB, F], F16, kind="ExternalInput").ap()
    invct_d = nc.dram_tensor("invct2", [1, 2 * NB], F32, kind="ExternalInput").ap()
    z_d = nc.dram_tensor("z", [B_LOC, T, CH * NB], F16, kind="ExternalOutput").ap()

    AluOp = mybir.AluOpType
    ActFn = mybir.ActivationFunctionType

    with tile.TileContext(nc) as tc:
        with (
            tc.tile_pool(name="const", bufs=1) as constp,
            tc.tile_pool(name="a", bufs=5) as ap_,
            tc.tile_pool(name="sq", bufs=2) as sqp,
            tc.tile_pool(name="eo", bufs=4) as eop,
            tc.tile_pool(name="xg", bufs=1) as xgp,
            tc.tile_pool(name="small", bufs=6) as smp,
            tc.tile_pool(name="per", bufs=1) as perp,
            tc.tile_pool(name="out", bufs=3) as outp,
            tc.tile_pool(name="psum", bufs=1, space="PSUM") as psp,
        ):
            # ---------------- constants to SBUF (scalar queue) ----------
            wt_sb = []
            for t, (lo, hi, _) in enumerate(TILES):
                w = constp.tile([TILE_ROWS[t], CH * (hi - lo)], F16,
                                tag=f"wt_{t}", name=f"wt_{t}")
                nc.scalar.dma_start(w[:], wt_d[t][:])
                wt_sb.append(w)
            mbias_sb = constp.tile([63, 4096], F16, tag="mbias")
            nc.scalar.dma_start(mbias_sb[:], mbias_d[:])
            ind_sb, indT_sb = [], []
            for g, (f0, P) in enumerate(FT):
                it = constp.tile([P, NB], F32, tag=f"ind_{g}", name=f"ind_{g}")
                nc.scalar.dma_start(it[:], ind_d[f0:f0 + P, :])
                ind_sb.append(it)
                jt = constp.tile([NB, P], F16, tag=f"indT_{g}",
                                 name=f"indT_{g}")
                nc.scalar.dma_start(jt[:], indT_d[:, f0:f0 + P])
                indT_sb.append(jt)
            invct_sb = constp.tile([1, 2 * NB], F32, tag="invct")
            nc.scalar.dma_start(invct_sb[:], invct_d[:])
            ident = constp.tile([1, 1], F32, tag="ident")
            nc.vector.memset(ident[:], 1.0)
            epsc = constp.tile([1, 1], F32, tag="epsc")
            nc.vector.memset(epsc[:], EPS)
            ones1 = constp.tile([1, 128], F16, tag="ones1")
            nc.vector.memset(ones1[:], 1.0)
            ones63 = constp.tile([63, 128], F16, tag="ones63")
            nc.vector.memset(ones63[:], 1.0)

            # persistent per-sample tiles
            xg = [[xgp.tile([TILE_ROWS[t], T], F16, tag=f"xg_{s}_{t}",
                            name=f"xg_{s}_{t}")
                   for t in range(N_XT)] for s in range(B_LOC)]
            v63 = [perp.tile([63, 1], F16, tag=f"v63_{s}", name=f"v63_{s}")
                   for s in range(B_LOC)]
            v63f = [perp.tile([63, 1], F32, tag=f"v63f_{s}",
                              name=f"v63f_{s}") for s in range(B_LOC)]
            sfrow = [[perp.tile([P, 1], F32, tag=f"sf_{s}_{g}",
                                name=f"sf_{s}_{g}")
                      for g, (f0, P) in enumerate(FT)] for s in range(B_LOC)]
            bias_sb = [perp.tile([128, 4096], F16, tag=f"bias_{s}",
                                 name=f"bias_{s}") for s in range(B_LOC)]
            v63rep = [perp.tile([63, 128], F16, tag=f"v63r_{s}",
                                name=f"v63r_{s}") for s in range(B_LOC)]

            # ---------------- body (repeatable for benchmarking) --------
            for _rep in range(repeat):
              # ---------------- prologue per sample ----------------
              for s in range(B_LOC):
                  mom = psp.tile([1, 2 * NB], F32, tag="bank0", name="mom")
                  stats = []
                  As = []
                  for g, (f0, P) in enumerate(FT):
                      A = ap_.tile([P, 2000], F32, tag="a")
                      nc.sync.dma_start(
                          A[:], x_d[s, f0:f0 + P].rearrange("p a b -> p (a b)"))
                      As.append(A)
                      stat = smp.tile([P, 2], F32, tag="stat")
                      stats.append(stat)
                      # Sxx: square on scalar engine with accumulate
                      Asq = sqp.tile([P, 2000], F32, tag="sq", name="Asq")
                      nc.scalar.activation(Asq[:], A[:], ActFn.Square,
                                           accum_out=stat[:, 1:2])
                      # Sx: plain reduce on vector
                      nc.vector.tensor_reduce(stat[:, 0:1], A[:],
                                              mybir.AxisListType.X, AluOp.add)
                      b0, b1 = FT_BANDS[g]
                      nc.tensor.matmul(mom[0:1, b0:b1], lhsT=stat[:, 0:1],
                                       rhs=ind_sb[g][:, b0:b1],
                                       start=True, stop=True)
                      nc.tensor.matmul(mom[0:1, NB + b0:NB + b1],
                                       lhsT=stat[:, 1:2],
                                       rhs=ind_sb[g][:, b0:b1],
                                       start=True, stop=True)

                  # moments -> s, -mu*s on partition 0
                  m2 = smp.tile([1, 2 * NB], F32, tag="m2")
                  nc.vector.tensor_tensor(m2[:], mom[:], invct_sb[:],
                                          AluOp.mult)   # [mu | ex2]
                  mu = m2[:, 0:NB]
                  var = smp.tile([1, NB], F32, tag="var")
                  nc.vector.tensor_tensor(var[:], mu, mu, AluOp.mult)
                  nc.vector.tensor_tensor(var[:], m2[:, NB:2 * NB], var[:],
                                          AluOp.subtract)
                  sd = smp.tile([1, NB], F32, tag="sd")
                  nc.scalar.activation(sd[:], var[:], ActFn.Sqrt,
                                       bias=epsc[:])
                  vrow = smp.tile([1, 64], F32, tag="vrow")
                  nc.vector.reciprocal(vrow[:, 0:NB], sd[:])         # s
                  tmp = smp.tile([1, NB], F32, tag="tmp")
                  nc.vector.tensor_tensor(tmp[:], mu, vrow[:, 0:NB],
                                          AluOp.mult)
                  nc.vector.tensor_scalar(vrow[:, NB:2 * NB], tmp[:], -1.0,
                                          None, AluOp.mult)          # -mu*s
                  nc.vector.memset(vrow[:, 62:63], 1.0)

                  v63p = psp.tile([63, 1], F32, tag="bank1", name="v63p")
                  nc.tensor.transpose(v63p[:], vrow[:, 0:63], ident[:])
                  nc.vector.tensor_copy(v63[s][:], v63p[:])
                  nc.vector.tensor_copy(v63f[s][:], v63p[:])

                  # per-f-row s scale (s_frow = indT^T @ s)
                  for g, (f0, P) in enumerate(FT):
                      sfp = psp.tile([P, 1], F32, tag=f"bank{2 + g}",
                                     name=f"sfp{g}")
                      nc.tensor.matmul(sfp[:], lhsT=indT_sb[g][:],
                                       rhs=v63[s][0:NB, :],
                                       start=True, stop=True)
                      nc.vector.tensor_copy(sfrow[s][g][:], sfp[:])

                  # bias table (bank-order cols), replicated over the 128
                  # t-partitions by the PE: one [128,512] matmul per bank
                  nc.vector.tensor_scalar(v63rep[s][:], ones63[:],
                                          v63f[s][:, 0:1], None, AluOp.mult)
                  for j in range(8):
                      bps = psp.tile([128, 512], F32, tag=f"bank{j}",
                                     name=f"bps{j}")
                      nc.tensor.matmul(bps[:], lhsT=v63rep[s][:],
                                       rhs=mbias_sb[:, j * 512:(j + 1) * 512],
                                       start=True, stop=True)
                      dst = bias_sb[s][:, j * 512:(j + 1) * 512]
                      if j < 5:
                          nc.vector.tensor_copy(dst, bps[:])
                      else:
                          nc.scalar.copy(dst, bps[:])

                  # de-interleave with s scaling, then slab DMAs
                  for g, (f0, P) in enumerate(FT):
                      Av = As[g][:].rearrange("p (t r) -> p r t", r=2)
                      E = eop.tile([P, T], F16, tag="eo", name="E")
                      O = eop.tile([P, T], F16, tag="eo", name="O")
                      nc.vector.tensor_scalar(E[:], Av[:, 0, :],
                                              sfrow[s][g][:, 0:1], None,
                                              AluOp.mult)
                      nc.scalar.activation(O[:], Av[:, 1, :], ActFn.Copy,
                                           scale=sfrow[s][g][:, 0:1])
                      for t in range(N_XT):
                          if TILE_FT[t] != g:
                              continue
                          _, _, fst = TILES[t]
                          r0 = fst - f0
                          e = ESUM[t]
                          nc.gpsimd.dma_start(xg[s][t][0:e, :],
                                              E[r0:r0 + e, :])
                          nc.gpsimd.dma_start(xg[s][t][e:2 * e, :],
                                              O[r0:r0 + e, :])

              # ---------------- main loop ----------------
              for s in range(B_LOC):
                  for (t0, M) in CHUNKS:
                      ob = outp.tile([128, CH * NB], F16, tag="ob")
                      ob_v = ob[0:M].rearrange("p (o i) -> p o i", o=CH, i=NB)
                      ps = [psp.tile([128, 512], F32, tag=f"bank{j}",
                                     name=f"ps{j}") for j in range(8)]
                      for j in PREFILL_BANKS:
                          # pre-load bias for scalar-drained banks
                          w = max(GROUPS[g][4] + (GROUPS[g][2] - GROUPS[g][1])
                                  * CH for g in range(len(GROUPS))
                                  if GROUPS[g][3] == j)
                          nc.tensor.matmul(
                              ps[j][0:M, 0:w], lhsT=ones1[0:1, 0:M],
                              rhs=bias_sb[s][0:1, j * 512:j * 512 + w],
                              start=True, stop=False, skip_group_check=True)
                      for gi, (t, blo, bhi, bank, bcol, wtcol) in \
                              enumerate(GROUPS):
                          nb = bhi - blo
                          n = nb * CH
                          rend = GREND[gi]
                          pre = bank in PREFILL_BANKS
                          nc.tensor.matmul(
                              ps[bank][0:M, bcol:bcol + n],
                              lhsT=xg[s][t][0:rend, t0:t0 + M],
                              rhs=wt_sb[t][0:rend, wtcol:wtcol + n],
                              start=not pre, stop=True)
                      # drains
                      for gi, (t, blo, bhi, bank, bcol, wtcol) in \
                              enumerate(GROUPS):
                          nb = bhi - blo
                          n = nb * CH
                          dst = ob_v[:, :, blo:bhi]
                          src = ps[bank][0:M, bcol:bcol + n].rearrange(
                              "p (o i) -> p o i", o=CH, i=nb)
                          eng = DRAIN_ENG[gi]
                          if eng == "s":
                              nc.scalar.copy(dst, src)
                          else:
                              bia = bias_sb[s][0:M,
                                               bank * 512 + bcol:
                                               bank * 512 + bcol + n]
                              bia = bia.rearrange("p (o i) -> p o i",
                                                  o=CH, i=nb)
                              e = nc.vector if eng == "v" else nc.gpsimd
                              e.tensor_tensor(dst, src, bia, AluOp.add)
                      nc.sync.dma_start(z_d[s, t0:t0 + M], ob[0:M, :])

    _NC_CACHE[key] = nc
    return nc


# ----------------------------------------------------------------------------
# Public entry point
# ----------------------------------------------------------------------------
def kernel(x, gn_w, gn_b, fc_w, fc_b):
    x = np.asarray(x, np.float32)
    gn_w = np.asarray(gn_w, np.float32)
    gn_b = np.asarray(gn_b, np.float32)
    fc_w = np.asarray(fc_w, np.float32)
    fc_b = np.asarray(fc_b, np.float32)

    wts, mbias, ind, indT, invct2 = _build_const_tables(gn_w, gn_b,
                                                        fc_w, fc_b)
    nc = build_bass()
    if not getattr(nc, "_waits_spilled", False):
        _spill_waits(nc)
        nc._waits_spilled = True

    in_maps = []
    for k in range(N_CORES):
        m = {"x": np.ascontiguousarray(x[k * B_LOC:(k + 1) * B_LOC]),
             "mbias": mbias, "ind": ind, "indT": indT, "invct2": invct2}
        for t in range(N_XT):
            m[f"wt{t}"] = wts[t]
        in_maps.append(m)
    res = run_bass_kernel_spmd(nc, in_maps, core_ids=list(range(N_CORES)))
    z16 = np.concatenate([r["z"] for r in res.results], axis=0)
    return z16.reshape(B_FULL, T, CH, NB).astype(np.float32)
